# revision 1
# baseline (speedup 1.0000x reference)
"""Trainium2 Bass kernel v2 for nn_FATMSparse (spiking Haar-wavelet network).

Channel-sharded over 8 cores (32 ch/core), partitions p = b*32 + c_local.
vs v1: LIF in u=2v form writing spikes directly into the padded conv tile;
Haar stages bf16 (exact small ints, 2x DVE); BN_fwd stats from spike
identities; single-bf16 conv weights with conv bias dropped (training-BN
invariant); block-diag matmul butterflied in PSUM; bn_stats everywhere.
"""
import sys

sys.path.insert(0, "/opt/trn_rl_repo")

import numpy as np

import concourse.bass as bass
import concourse.bacc as bacc
import concourse.tile as tile
from concourse import mybir
from concourse.bass_utils import run_bass_kernel_spmd

F32 = mybir.dt.float32
BF16 = mybir.dt.bfloat16
AX = mybir.AxisListType
OP = mybir.AluOpType
AF = mybir.ActivationFunctionType

T, B, C, H, W = 4, 4, 256, 32, 32
CL = 32
NCORES = 8
P = 128
FT = H * W            # 1024
F = T * FT            # 4096
Hh, Wh = 16, 16
INV_SQRT2 = float(np.float32(1.0 / np.sqrt(2.0)))
SQRT2B = float(np.float32(2.0) * np.float32(INV_SQRT2))
TAUS = [0.01, 0.02, 0.02, 0.05]

RSPL = 20             # LIF split: DVE rows [0:RSPL), Pool rows [RSPL:32)
HSPL = 512            # H-phase conv-term split (DVE below, Pool above)


def build_module(kstage=9):
    nc = bacc.Bacc("TRN2", target_bir_lowering=False, debug=False)

    def din(name, shape, dt):
        return nc.dram_tensor(name, shape, dt, kind="ExternalInput").ap()

    xin_d = din("xin", [P, F], F32)
    w1_d = din("w1blk", [P, P], BF16)
    w2_d = din("w2blk", [P, 9 * P], BF16)
    wk_d = din("wkblk", [P, 4 * P], F32)
    selc_d = din("selc", [P, CL], F32)
    selb_d = din("selb", [CL, P], F32)
    bnp_d = din("bnp", [CL, 21], F32)
    thr_d = din("thrv", [P, 16], F32)
    out_d = nc.dram_tensor("out", [P, F], F32, kind="ExternalOutput").ap()

    with tile.TileContext(nc) as tc:
        _emit(tc, nc, kstage, xin_d, w1_d, w2_d, wk_d, selc_d, selb_d,
              bnp_d, thr_d, out_d)
    nc.finalize()
    return nc


def _emit(tc, nc, KSTAGE, xin_d, w1_d, w2_d, wk_d, selc_d, selb_d,
          bnp_d, thr_d, out_d):
    import contextlib

    ctx = contextlib.ExitStack()
    consts = ctx.enter_context(tc.tile_pool(name="consts", bufs=1))
    big = ctx.enter_context(tc.tile_pool(name="big", bufs=1))
    scr = ctx.enter_context(tc.tile_pool(name="scr", bufs=2))
    small = ctx.enter_context(tc.tile_pool(name="small", bufs=1))
    psA = ctx.enter_context(tc.tile_pool(name="psA", bufs=2, space="PSUM"))
    psC = ctx.enter_context(tc.tile_pool(name="psC", bufs=4, space="PSUM"))

    # ---- constants ----
    w1_sb = consts.tile([P, P], BF16, tag="w1")
    nc.scalar.dma_start(out=w1_sb, in_=w1_d[:])
    w2_sb = consts.tile([P, 9, P], BF16, tag="w2")
    nc.scalar.dma_start(out=w2_sb, in_=w2_d[:].rearrange("p (k n) -> p k n", k=9))
    wk_sb = consts.tile([P, 4, P], F32, tag="wk")
    nc.scalar.dma_start(out=wk_sb, in_=wk_d[:].rearrange("p (k n) -> p k n", k=4))
    selc_sb = consts.tile([P, CL], F32, tag="selc")
    nc.gpsimd.dma_start(out=selc_sb, in_=selc_d[:])
    selb_sb = consts.tile([CL, P], F32, tag="selb")
    nc.gpsimd.dma_start(out=selb_sb, in_=selb_d[:])
    bnp_sb = consts.tile([CL, 21], F32, tag="bnp")
    nc.gpsimd.dma_start(out=bnp_sb, in_=bnp_d[:])
    thr_sb = consts.tile([P, 16], F32, tag="thrv")
    nc.gpsimd.dma_start(out=thr_sb, in_=thr_d[:])

    # force the act table containing {identity,copy,square,rsqrt} early
    sqd = consts.tile([P, 1], F32, tag="sqd")
    nc.vector.memset(sqd[:], 1.0)
    nc.scalar.activation(out=sqd[:], in_=sqd[:], func=AF.Sqrt)

    # diagonal mask (j == p) and ones row for the H-phase PE combine
    dmask = consts.tile([P, P], BF16, tag="dmask")
    dmi = consts.tile([P, P], F32, tag="dmi")
    nc.gpsimd.iota(dmi[:].bitcast(mybir.dt.int32), pattern=[[1, P]], base=0,
                   channel_multiplier=-1)
    nc.gpsimd.tensor_single_scalar(out=dmask[:], in_=dmi[:].bitcast(mybir.dt.int32),
                                   scalar=0, op=OP.is_equal)
    ones5 = consts.tile([P, 512], BF16, tag="ones5")
    nc.gpsimd.memset(ones5[:], 1.0)

    # ---- big tiles ----
    xin = big.tile([P, T, H, W], F32, tag="xin")
    u = big.tile([P, H, W], F32, tag="u")
    um = big.tile([P, H - RSPL, W], F32, tag="um")
    spad = big.tile([P, T, H + 2, W + 2], BF16, tag="spad")
    ulo = big.tile([P, T, H, Wh], BF16, tag="ulo")
    uhi = big.tile([P, T, H, Wh], BF16, tag="uhi")
    pq = big.tile([P, 4, T, Hh, Wh], BF16, tag="pq")
    cf = big.tile([P, 4, T, 256], BF16, tag="cf")
    c1 = big.tile([P, T, H, W], BF16, tag="c1")
    c2 = big.tile([P, T, H, W], BF16, tag="c2")
    habcd = big.tile([P, 4, T, 256], BF16, tag="habcd")   # raw + gb biases
    recb = big.tile([P, T, H, W], BF16, tag="recb")
    wkb = big.tile([P, 8, P], BF16, tag="wkb")
    zt = big.tile([P, 2, T, 256], F32, tag="zt")
    zz = big.tile([P, 2, T, 256], F32, tag="zz")
    cbt = big.tile([P, 4, T, 256], F32, tag="cbt")

    # ---- small tiles ----
    sp1 = small.tile([P, 4], F32, tag="sp1")
    st6c = small.tile([P, 8, 6], F32, tag="st6c")         # c1 psum stats
    st6e = small.tile([P, 8, 6], F32, tag="st6e")         # E psum stats
    st6d = small.tile([P, 16, 6], F32, tag="st6d")        # band stats
    c2s = small.tile([P, 8, 2], F32, tag="c2s")           # c2 sum/sumsq cells
    c1s = small.tile([P, 8, 2], F32, tag="c1s")           # c1 sum/sumsq cells
    sd = small.tile([P, 2, 16], F32, tag="sd")            # [S_cb, S2_cb] per band,t
    mek = small.tile([P, 16], F32, tag="mek")
    pt1 = small.tile([P, 4], F32, tag="pt1")
    pt2 = small.tile([P, 8], F32, tag="pt2")
    pt34 = small.tile([P, 6], F32, tag="pt34")
    ab1 = small.tile([P, 4], F32, tag="ab1")
    ab2 = small.tile([P, 8], F32, tag="ab2")
    ab3 = small.tile([P, 4], F32, tag="ab3")
    bdb = small.tile([P, 4], F32, tag="bdb")
    gb = small.tile([P, 4], F32, tag="gb")
    sq1 = small.tile([P, 1], F32, tag="sq1")
    w16 = small.tile([P, 16], F32, tag="w16")
    w16b = small.tile([P, 16], F32, tag="w16b")
    shc = small.tile([P, 8, 2], F32, tag="shc")

    # ========= phase A: load x, LIF (u = 2v) =========
    xinv = xin[:].rearrange("p t h w -> p (t h w)")
    NSP = RSPL * W
    for t in range(T):
        q = nc.sync if t < 2 else nc.gpsimd
        q.dma_start(out=xinv[:, t * FT:t * FT + NSP],
                    in_=xin_d[:, t * FT:t * FT + NSP])
        q.dma_start(out=xinv[:, t * FT + NSP:(t + 1) * FT],
                    in_=xin_d[:, t * FT + NSP:(t + 1) * FT])
    nc.gpsimd.memset(spad[:, :, 0, :], 0.0)
    nc.gpsimd.memset(spad[:, :, H + 1, :], 0.0)
    nc.gpsimd.memset(spad[:, :, :, 0], 0.0)
    nc.gpsimd.memset(spad[:, :, :, W + 1], 0.0)

    spi = spad[:, :, 1:H + 1, 1:W + 1]
    for t in range(T):
        xd_ = xin[:, t, 0:RSPL, :]
        xp_ = xin[:, t, RSPL:H, :]
        ud_ = u[:, 0:RSPL, :]
        up_ = u[:, RSPL:H, :]
        sd_ = spad[:, t, 1:RSPL + 1, 1:W + 1]
        sp_ = spad[:, t, RSPL + 1:H + 1, 1:W + 1]
        if t == 0:
            nc.vector.tensor_copy(ud_, xd_)
        else:
            nc.vector.scalar_tensor_tensor(
                out=ud_, in0=ud_, scalar=0.5, in1=xd_, op0=OP.mult, op1=OP.add)
        nc.vector.tensor_scalar(
            out=sd_, in0=ud_, scalar1=2.0, scalar2=0.0,
            op0=OP.is_ge, op1=OP.add)
        if t < T - 1:
            nc.vector.scalar_tensor_tensor(
                out=ud_, in0=ud_, scalar=2.0, in1=ud_, op0=OP.is_lt, op1=OP.mult)
        if t == 0:
            nc.gpsimd.tensor_copy(up_, xp_)
        else:
            nc.gpsimd.tensor_scalar_mul(up_, up_, 0.5)
            nc.gpsimd.tensor_add(up_, up_, xp_)
        nc.gpsimd.tensor_single_scalar(
            out=sp_, in_=up_, scalar=2.0, op=OP.is_ge)
        if t < T - 1:
            nc.gpsimd.tensor_single_scalar(
                out=um[:], in_=up_, scalar=2.0, op=OP.is_lt)
            nc.gpsimd.tensor_mul(up_, up_, um[:])

    if KSTAGE == 1:
        ov = big.tile([P, T, H, W], F32, tag="xin2")
        nc.vector.tensor_copy(ov[:], spi)
        nc.sync.dma_start(out=out_d[:], in_=ov[:].rearrange("p t h w -> p (t h w)"))
        ctx.close()
        return

    # ========= conv path: PE matmuls + copies/stats =========
    c1v = c1[:].rearrange("p t h w -> p (t h w)")
    c2v = c2[:].rearrange("p t h w -> p (t h w)")

    def conv_chunk(k):
        t, hs = k // 2, (k % 2) * 16
        p1 = psC.tile([P, 512], F32, tag="psc")
        p2 = psC.tile([P, 512], F32, tag="psc")
        nc.tensor.matmul(p1, w1_sb[:], spad[:, t, hs + 1:hs + 17, 1:W + 1],
                         start=True, stop=True)
        for i, (dy, dx) in enumerate([(a, b) for a in range(3) for b in range(3)]):
            nc.tensor.matmul(p2, w2_sb[:, i],
                             spad[:, t, hs + dy:hs + dy + 16, dx:dx + 32],
                             start=(i == 0), stop=(i == 8))
        # c1: Act plain copy (stats computed later / analytically)
        nc.scalar.activation(out=c1v[:, k * 512:(k + 1) * 512], in_=p1,
                             func=AF.Identity)
        # c2: DVE copy+accum + Act square+accum
        nc.vector.tensor_scalar(
            out=c2v[:, k * 512:(k + 1) * 512], in0=p2, scalar1=1.0, scalar2=0.0,
            op0=OP.mult, op1=OP.add, accum_out=c2s[:, k, 0:1])
        sqs = scr.tile([P, 512], F32, tag="sqscr")
        nc.scalar.activation(out=sqs[:], in_=p2, func=AF.Square,
                             accum_out=c2s[:, k, 1:2])

    for k in range(4):
        conv_chunk(k)

    if KSTAGE == 2:
        for k in range(4, 8):
            conv_chunk(k)
        ov = big.tile([P, T, FT], F32, tag="xin2")
        ovv = ov[:].rearrange("p t f -> p (t f)")
        nc.vector.tensor_copy(ovv, c2v[:])
        nc.sync.dma_start(out=out_d[:], in_=ovv)
        ctx.close()
        return

    # ========= phases B+C per-t: Haar W then H (bf16, Pool) + fwd stats =========
    sacc = small.tile([P, 3, T], F32, tag="sacc")
    wsta = scr.tile([P, 2, H, Wh], BF16, tag="wsta")
    wstb = scr.tile([P, 2, H, Wh], BF16, tag="wstb")
    wstc = scr.tile([P, 2, H, Wh], BF16, tag="wstc")
    for t in range(T):
        se = spad[:, t, 1:H + 1, 1:W + 1:2]
        so = spad[:, t, 1:H + 1, 2:W + 2:2]
        nc.gpsimd.tensor_add(ulo[:, t], se, so)
        nc.gpsimd.tensor_sub(uhi[:, t], se, so)
        nc.vector.tensor_scalar(out=wsta[:, t % 2], in0=ulo[:, t], scalar1=1.0,
                                scalar2=0.0, op0=OP.mult, op1=OP.add,
                                accum_out=sacc[:, 0, t:t + 1])
        nc.vector.tensor_scalar(out=wstb[:, t % 2], in0=uhi[:, t], scalar1=1.0,
                                scalar2=0.0, op0=OP.mult, op1=OP.add,
                                accum_out=sacc[:, 1, t:t + 1])
        nc.vector.tensor_scalar(out=wstc[:, t % 2], in0=ulo[:, t], scalar1=2.0,
                                scalar2=0.0, op0=OP.is_equal, op1=OP.add,
                                accum_out=sacc[:, 2, t:t + 1])
        ue, uo = ulo[:, t, 0::2, :], ulo[:, t, 1::2, :]
        he, ho = uhi[:, t, 0::2, :], uhi[:, t, 1::2, :]
        nc.gpsimd.tensor_add(pq[:, 0, t], ue, uo)
        nc.gpsimd.tensor_sub(pq[:, 1, t], ue, uo)
        nc.gpsimd.tensor_add(pq[:, 2, t], he, ho)
        nc.gpsimd.tensor_sub(pq[:, 3, t], he, ho)
    for j in range(3):
        nc.vector.tensor_reduce(out=sp1[:, j:j + 1], in_=sacc[:, j],
                                axis=AX.X, op=OP.add)

    # fwd BN: S1lo=Sulo, S1hi=Suhi, S2lo=Sulo+2*Sm2, S2hi=Sulo-2*Sm2
    nc.vector.tensor_copy(pt1[:, 0:2], sp1[:, 0:2])
    nc.vector.tensor_scalar(out=sp1[:, 3:4], in0=sp1[:, 2:3], scalar1=2.0,
                            scalar2=0.0, op0=OP.mult, op1=OP.add)
    nc.vector.tensor_add(pt1[:, 2:3], pt1[:, 0:1], sp1[:, 3:4])   # S2lo
    nc.vector.tensor_sub(pt1[:, 3:4], pt1[:, 0:1], sp1[:, 3:4])   # S2hi
    st1 = psA.tile([CL, 4], F32, tag="psa")
    nc.tensor.matmul(st1, selc_sb[:], pt1[:], start=True, stop=True)
    # Sc1 = W1 @ Ss (1x1 conv sum identity; Ss = Sulo)
    ssb = small.tile([P, 1], BF16, tag="ssb")
    nc.vector.tensor_copy(ssb[:], sp1[:, 0:1])
    sc1p = psA.tile([P, 1], F32, tag="psa")
    nc.tensor.matmul(sc1p, w1_sb[:], ssb[:], start=True, stop=True)
    nc.vector.tensor_copy(sp1[:, 3:4], sc1p)
    sb1 = small.tile([CL, 4], F32, tag="sb1")
    nc.vector.tensor_copy(sb1[:], st1)
    w32 = small.tile([CL, 12], F32, tag="w32")
    _bn_small(nc, sb1[:, 0:2], sb1[:, 2:4],
              n=8192.0, eps=2e-5,
              g=bnp_sb[:, 0:2], b=bnp_sb[:, 2:4],
              outA=w32[:, 0:2], outB=w32[:, 2:4], w=w32[:, 4:12])
    bc1 = small.tile([CL, 4], F32, tag="bc1")
    nc.vector.tensor_scalar_mul(bc1[:, 0:2], w32[:, 0:2], INV_SQRT2)
    nc.vector.tensor_scalar_mul(bc1[:, 2:4], w32[:, 2:4], SQRT2B)
    bp1 = psA.tile([P, 4], F32, tag="psa")
    nc.tensor.matmul(bp1, selb_sb[:], bc1[:], start=True, stop=True)
    nc.vector.tensor_copy(ab1[:], bp1)

    # ========= phase D: bands =========
    # bands: 0 LL(+B, A0) | 1 HL(A0, int-gate) | 2 LH(+B, A1) | 3 HH(A1, int-gate)
    pqv = pq[:].rearrange("p k t u w -> p k t (u w)")
    cfv = cf[:].rearrange("p k t x -> p k (t x)")
    # per-band r2 = 0.25/A^2 and thr3 = 256*tau/A^2 for int-gate bands (1,3)
    dgx = small.tile([P, 8], F32, tag="dgx")   # [A2sq(2), iA2(2), r2(2), thr3(2)]
    nc.vector.tensor_mul(dgx[:, 0:2], ab1[:, 0:2], ab1[:, 0:2])
    nc.vector.reciprocal(dgx[:, 2:4], dgx[:, 0:2])
    nc.vector.tensor_scalar_mul(dgx[:, 4:6], dgx[:, 2:4], 0.25)
    nc.vector.tensor_mul(dgx[:, 6:7], thr_sb[:, 4:5], dgx[:, 2:3])
    nc.vector.tensor_mul(dgx[:, 7:8], thr_sb[:, 12:13], dgx[:, 3:4])
    amek = small.tile([P, 16], F32, tag="amek")
    for bi in range(4):
        ci = bi // 2
        cb = cbt[:, bi]
        cbb = cbt[:, bi].rearrange("p t x -> p (t x)")
        a_ap = ab1[:, ci:ci + 1]
        b_ap = ab1[:, 2 + ci:3 + ci]
        b4 = slice(4 * bi, 4 * bi + 4)
        pqf = pqv[:, bi].rearrange("p t x -> p (t x)")
        if bi % 2 == 0:
            # biased band: z on Act, zz Pool, cb = (zz>=.25)*z on DVE
            zv = zt[:, bi % 2].rearrange("p t x -> p (t x)")
            zzv = zz[:, bi % 2].rearrange("p t x -> p (t x)")
            nc.scalar.activation(out=zv, in_=pqf, func=AF.Identity,
                                 bias=b_ap, scale=a_ap)
            nc.gpsimd.tensor_mul(zzv, zv, zv)
            nc.vector.scalar_tensor_tensor(
                out=cbb, in0=zzv, scalar=0.25, in1=zv,
                op0=OP.is_ge, op1=OP.mult)
            cbs = cb
        else:
            # int-gate band: pg2 = pq^2 (Pool, exact bf16), g = pg2 >= r2 (DVE),
            # cb' = pq*g (Pool, exact bf16 ints)
            pg2 = scr.tile([P, T, 256], F32, tag="pg2")
            gg = scr.tile([P, T, 256], BF16, tag="gg")
            cbi = scr.tile([P, T, 256], BF16, tag="cbi")
            nc.gpsimd.tensor_mul(pg2[:].rearrange("p t x -> p (t x)"), pqf, pqf)
            nc.vector.tensor_scalar(
                out=gg[:].rearrange("p t x -> p (t x)"),
                in0=pg2[:].rearrange("p t x -> p (t x)"),
                scalar1=dgx[:, 4 + ci:5 + ci], scalar2=0.0,
                op0=OP.is_ge, op1=OP.add)
            nc.gpsimd.tensor_mul(cbi[:].rearrange("p t x -> p (t x)"),
                                 gg[:].rearrange("p t x -> p (t x)"), pqf)
            cbs = cbi
        if bi % 2 == 0:
            # E stats via DVE bn_stats per t; extract; cf on Pool
            for t in range(T):
                nc.vector.bn_stats(out=st6d[:, 4 * bi + t], in_=cbs[:, t])
            cs = st6d[:, 4 * bi:4 * bi + 4]
            nc.vector.tensor_add(sd[:, 0, b4], cs[:, :, 1], cs[:, :, 4])
            nc.vector.tensor_scalar_mul(sd[:, 0, b4], sd[:, 0, b4], 128.0)
            nc.vector.tensor_mul(w16[:, b4], cs[:, :, 1], cs[:, :, 1])
            nc.vector.tensor_mul(w16[:, 8:12], cs[:, :, 4], cs[:, :, 4])
            nc.vector.tensor_add(w16[:, b4], w16[:, b4], w16[:, 8:12])
            nc.vector.tensor_scalar_mul(w16[:, b4], w16[:, b4], 128.0)
            nc.vector.tensor_add(sd[:, 1, b4], cs[:, :, 2], cs[:, :, 5])
            nc.vector.tensor_add(sd[:, 1, b4], sd[:, 1, b4], w16[:, b4])
            nc.vector.tensor_tensor(out=mek[:, b4], in0=sd[:, 1, b4],
                                    in1=thr_sb[:, b4], op=OP.is_gt)
            nc.vector.tensor_copy(amek[:, b4], mek[:, b4])
            for t in range(T):
                nc.gpsimd.tensor_scalar_mul(cf[:, bi, t], cbs[:, t],
                                            mek[:, 4 * bi + t:4 * bi + t + 1])
        else:
            # int band: Spg via DVE bf16 ts+accum; S2pg via Act square+accum
            for t in range(T):
                wbt = scr.tile([P, 256], BF16, tag="dsqb")
                nc.vector.tensor_scalar(
                    out=wbt[:], in0=cbs[:, t], scalar1=1.0, scalar2=0.0,
                    op0=OP.mult, op1=OP.add,
                    accum_out=sd[:, 0, 4 * bi + t:4 * bi + t + 1])
                sqa = scr.tile([P, 256], F32, tag="dsq")
                nc.scalar.activation(out=sqa[:], in_=cbs[:, t], func=AF.Square,
                                     accum_out=sd[:, 1, 4 * bi + t:4 * bi + t + 1])
            nc.vector.tensor_scalar(
                out=mek[:, b4], in0=sd[:, 1, b4],
                scalar1=dgx[:, 6 + ci:7 + ci], scalar2=0.0,
                op0=OP.is_gt, op1=OP.add)
            nc.vector.tensor_scalar(
                out=amek[:, b4], in0=mek[:, b4], scalar1=a_ap, scalar2=0.0,
                op0=OP.mult, op1=OP.add)
            for t in range(T):
                nc.gpsimd.tensor_scalar_mul(cf[:, bi, t], cbs[:, t],
                                            amek[:, 4 * bi + t:4 * bi + t + 1])

    for k in range(4, 8):
        conv_chunk(k)

    if KSTAGE == 3:
        ov = big.tile([P, T, FT], F32, tag="xin2")
        ovv = ov[:].rearrange("p t f -> p (t f)")
        nc.vector.tensor_copy(ovv, pq[:].rearrange("p k t u w -> p (k t u w)"))
        nc.sync.dma_start(out=out_d[:], in_=ovv)
        ctx.close()
        return

    # BN_mul stats: S1_b = sum_t amek*S_raw
    nc.vector.tensor_mul(w16[:], amek[:], sd[:, 0])
    nc.vector.tensor_mul(w16b[:], mek[:], sd[:, 1])
    for bi in (1, 3):
        nc.vector.tensor_scalar(
            out=w16b[:, 4 * bi:4 * bi + 4], in0=w16b[:, 4 * bi:4 * bi + 4],
            scalar1=dgx[:, bi // 2:bi // 2 + 1], scalar2=0.0,
            op0=OP.mult, op1=OP.add)
    for bi in range(4):
        nc.vector.tensor_reduce(out=pt2[:, bi:bi + 1], in_=w16[:, 4 * bi:4 * bi + 4],
                                axis=AX.X, op=OP.add)
        nc.vector.tensor_reduce(out=pt2[:, 4 + bi:5 + bi],
                                in_=w16b[:, 4 * bi:4 * bi + 4], axis=AX.X, op=OP.add)
    # PE warm-up: ramp the tensor engine while D finishes (no D deps)
    wup = psA.tile([P, 128], F32, tag="wup")
    for _ in range(64):
        nc.tensor.matmul(wup, wk_sb[:, 0], wk_sb[:, 1], start=True, stop=True)
    st2 = psA.tile([CL, 8], F32, tag="psa")
    nc.tensor.matmul(st2, selc_sb[:], pt2[:], start=True, stop=True)
    sb2 = small.tile([CL, 8], F32, tag="sb2")
    nc.vector.tensor_copy(sb2[:], st2)
    w32b = small.tile([CL, 24], F32, tag="w32b")
    _bn_small(nc, sb2[:, 0:4], sb2[:, 4:8],
              n=4096.0, eps=1e-5,
              g=bnp_sb[:, 4:8], b=bnp_sb[:, 8:12],
              outA=w32b[:, 0:4], outB=w32b[:, 4:8], w=w32b[:, 8:24])
    bp2 = psA.tile([P, 8], F32, tag="psa")
    nc.tensor.matmul(bp2, selb_sb[:], w32b[:, 0:8], start=True, stop=True)
    nc.vector.tensor_copy(ab2[:], bp2)

    if KSTAGE == 4:
        ov = big.tile([P, T, FT], F32, tag="xin2")
        ovv = ov[:].rearrange("p t f -> p (t f)")
        nc.vector.tensor_copy(ovv, cf[:].rearrange("p k t x -> p (k t x)"))
        nc.sync.dma_start(out=out_d[:], in_=ovv)
        ctx.close()
        return

    # ========= phase E: block-diag matmul butterflied in PSUM =========
    for b_ in range(4):
        nc.gpsimd.tensor_scalar_mul(wkb[:, b_], wk_sb[:, b_], ab2[:, b_:b_ + 1])
    for b_ in (1, 3):
        nc.gpsimd.tensor_scalar_mul(wkb[:, 4 + b_], wkb[:, b_], -1.0)
    bb = psA.tile([P, 4], F32, tag="psa")
    for b_ in range(4):
        nc.tensor.matmul(bb[:, b_:b_ + 1], wk_sb[:, b_], ab2[:, 4 + b_:5 + b_],
                         start=True, stop=True)
    nc.vector.tensor_copy(bdb[:], bb)
    nc.vector.tensor_add(gb[:, 0:1], bdb[:, 0:1], bdb[:, 1:2])   # hap
    nc.vector.tensor_sub(gb[:, 1:2], bdb[:, 0:1], bdb[:, 1:2])   # ham
    nc.vector.tensor_add(gb[:, 2:3], bdb[:, 2:3], bdb[:, 3:4])   # hbp
    nc.vector.tensor_sub(gb[:, 3:4], bdb[:, 2:3], bdb[:, 3:4])   # hbm

    hv = habcd[:].rearrange("p k t x -> p k (t x)")
    grp_bands = [(0, 1, False), (0, 1, True), (2, 3, False), (2, 3, True)]
    for g, (blo, bhi, neg) in enumerate(grp_bands):
        for ck in range(2):
            pe_ = psC.tile([P, 512], F32, tag="psc")
            nc.tensor.matmul(pe_, wkb[:, blo],
                             cfv[:, blo, ck * 512:(ck + 1) * 512],
                             start=True, stop=False)
            nc.tensor.matmul(pe_, wkb[:, bhi + (4 if neg else 0)],
                             cfv[:, bhi, ck * 512:(ck + 1) * 512],
                             start=False, stop=True)
            nc.vector.bn_stats(out=st6e[:, 2 * g + ck], in_=pe_)
            nc.scalar.activation(out=hv[:, g, ck * 512:(ck + 1) * 512], in_=pe_,
                                 func=AF.Identity, bias=gb[:, g:g + 1], scale=1.0)

    # ---- BN_inv + conv stats ----
    _sum_sumsq(nc, st6e, shc, w16)          # shc[:, chunk, 0]=S, [:,chunk,1]=S2
    sg = shc[:].rearrange("p (g c) o -> p g c o", g=4)
    nc.vector.tensor_add(w16b[:, 0:4], sg[:, :, 0, 0], sg[:, :, 1, 0])  # Sh_g
    nc.vector.tensor_add(w16b[:, 4:8], sg[:, :, 0, 1], sg[:, :, 1, 1])  # Sh2_g
    # Sfull = Sh + 1024*gb ; S2full = Sh2 + 2*gb*Sh + 1024*gb^2
    nc.vector.tensor_scalar(out=w16[:, 0:4], in0=gb[:], scalar1=1024.0,
                            scalar2=0.0, op0=OP.mult, op1=OP.add)
    nc.vector.tensor_add(w16b[:, 8:12], w16b[:, 0:4], w16[:, 0:4])      # Sfull
    nc.vector.tensor_mul(w16[:, 4:8], gb[:], w16b[:, 0:4])
    nc.vector.tensor_scalar(out=w16[:, 4:8], in0=w16[:, 4:8], scalar1=2.0,
                            scalar2=0.0, op0=OP.mult, op1=OP.add)
    nc.vector.tensor_mul(w16[:, 8:12], gb[:], gb[:])
    nc.vector.tensor_scalar(out=w16[:, 8:12], in0=w16[:, 8:12], scalar1=1024.0,
                            scalar2=0.0, op0=OP.mult, op1=OP.add)
    nc.vector.tensor_add(w16b[:, 4:8], w16b[:, 4:8], w16[:, 4:8])
    nc.vector.tensor_add(w16b[:, 4:8], w16b[:, 4:8], w16[:, 8:12])      # S2full
    nc.vector.tensor_add(pt34[:, 0:1], w16b[:, 8:9], w16b[:, 9:10])
    nc.vector.tensor_scalar(out=pt34[:, 0:1], in0=pt34[:, 0:1], scalar1=2.0,
                            scalar2=0.0, op0=OP.mult, op1=OP.add)      # S1_rec
    nc.vector.tensor_reduce(out=pt34[:, 3:4], in_=w16b[:, 4:8], axis=AX.X, op=OP.add)
    nc.vector.tensor_scalar(out=pt34[:, 3:4], in0=pt34[:, 3:4], scalar1=2.0,
                            scalar2=0.0, op0=OP.mult, op1=OP.add)      # S2_rec
    # conv1 stats: sum analytic (sp1[3]); sumsq from bf16 sbuf copies
    nc.vector.tensor_copy(pt34[:, 1:2], sp1[:, 3:4])
    c1sq = scr.tile([P, T, H, W], BF16, tag="c1sq")
    nc.gpsimd.tensor_mul(c1sq[:], c1[:], c1[:])
    c1qv = c1sq[:].rearrange("p t h w -> p (t h w)")
    for q in range(4):
        nc.vector.tensor_scalar(out=c1qv[:, q * FT:(q + 1) * FT],
                                in0=c1qv[:, q * FT:(q + 1) * FT],
                                scalar1=1.0, scalar2=0.0, op0=OP.mult, op1=OP.add,
                                accum_out=c1s[:, q, 1:2])
    nc.vector.tensor_reduce(out=pt34[:, 4:5], in_=c1s[:, 0:4, 1], axis=AX.X, op=OP.add)
    # conv2 stats from cells
    nc.vector.tensor_reduce(out=pt34[:, 2:3], in_=c2s[:, :, 0], axis=AX.X, op=OP.add)
    nc.vector.tensor_reduce(out=pt34[:, 5:6], in_=c2s[:, :, 1], axis=AX.X, op=OP.add)

    st3 = psA.tile([CL, 6], F32, tag="psa")
    nc.tensor.matmul(st3, selc_sb[:], pt34[:], start=True, stop=True)
    sb3 = small.tile([CL, 6], F32, tag="sb3")
    nc.vector.tensor_copy(sb3[:], st3)
    w32c = small.tile([CL, 18], F32, tag="w32c")
    _bn_small(nc, sb3[:, 0:3], sb3[:, 3:6],
              n=16384.0, eps=bnp_sb[:, 18:21],
              g=bnp_sb[:, 12:15], b=bnp_sb[:, 15:18],
              outA=w32c[:, 0:3], outB=w32c[:, 3:6], w=w32c[:, 6:18])
    bc3 = small.tile([CL, 4], F32, tag="bc3")
    nc.vector.tensor_copy(bc3[:, 0:1], w32c[:, 0:1])
    nc.vector.tensor_reduce(out=bc3[:, 1:2], in_=w32c[:, 3:6], axis=AX.X, op=OP.add)
    nc.vector.tensor_copy(bc3[:, 2:4], w32c[:, 1:3])
    bp3 = psA.tile([P, 4], F32, tag="psa")
    nc.tensor.matmul(bp3, selb_sb[:], bc3[:], start=True, stop=True)
    nc.vector.tensor_copy(ab3[:], bp3)

    if KSTAGE == 5:
        ov = big.tile([P, T, FT], F32, tag="xin2")
        ovv = ov[:].rearrange("p t f -> p (t f)")
        nc.vector.tensor_copy(ovv, habcd[:].rearrange("p k t x -> p (k t x)"))
        nc.sync.dma_start(out=out_d[:], in_=ovv)
        ctx.close()
        return

    # ========= phase F: inverse-Haar butterfly (bf16) =========
    hap, ham, hbp, hbm = (habcd[:, k].rearrange("p t (u w) -> p t u w", u=16)
                          for k in range(4))
    nc.gpsimd.tensor_add(recb[:, :, 0::2, 0::2], hap, hbp)
    nc.gpsimd.tensor_sub(recb[:, :, 0::2, 1::2], hap, hbp)
    nc.gpsimd.tensor_add(recb[:, :, 1::2, 0::2], ham, hbm)
    nc.gpsimd.tensor_sub(recb[:, :, 1::2, 1::2], ham, hbm)

    # ========= phase H: PE diag-matmul combine =========
    # out = diag(A_r)@rec + diag(A1)@c1 + diag(A2)@c2 + diag(Btot)@ones
    dg = big.tile([P, 4, P], BF16, tag="dg")
    nc.gpsimd.tensor_scalar_mul(dg[:, 0], dmask[:], ab3[:, 0:1])
    nc.gpsimd.tensor_scalar_mul(dg[:, 1], dmask[:], ab3[:, 2:3])
    nc.gpsimd.tensor_scalar_mul(dg[:, 2], dmask[:], ab3[:, 3:4])
    nc.gpsimd.tensor_scalar_mul(dg[:, 3], dmask[:], ab3[:, 1:2])
    ov = big.tile([P, T, FT], F32, tag="xin2")
    ovv = ov[:].rearrange("p t f -> p (t f)")
    recv = recb[:].rearrange("p t h w -> p (t h w)")
    dmaq = [nc.sync, nc.scalar, nc.gpsimd, nc.sync, nc.scalar, nc.gpsimd,
            nc.sync, nc.scalar]
    for ck in range(8):
        a_, b_ = ck * 512, (ck + 1) * 512
        ph = psC.tile([P, 512], F32, tag="psc")
        nc.tensor.matmul(ph, dg[:, 0], recv[:, a_:b_], start=True, stop=False)
        nc.tensor.matmul(ph, dg[:, 1], c1v[:, a_:b_], start=False, stop=False)
        if ck % 2 == 0:
            nc.tensor.matmul(ph, dg[:, 2], c2v[:, a_:b_], start=False, stop=False)
            nc.tensor.matmul(ph, dg[:, 3], ones5[:], start=False, stop=True)
            nc.vector.tensor_scalar(out=ovv[:, a_:b_], in0=ph, scalar1=1.0,
                                    scalar2=0.0, op0=OP.mult, op1=OP.add)
        else:
            nc.tensor.matmul(ph, dg[:, 2], c2v[:, a_:b_], start=False, stop=True)
            nc.scalar.activation(out=ovv[:, a_:b_], in_=ph, func=AF.Identity,
                                 bias=ab3[:, 1:2])
        dmaq[ck].dma_start(out=out_d[:, a_:b_], in_=ovv[:, a_:b_])
    ctx.close()


def _sum_sumsq(nc, st6, shc, w16):
    """shc[:, k, 0] = sum, shc[:, k, 1] = sumsq from bn_stats 6-tuples."""
    cnt = st6[:, :, 0::3]
    mu_ = st6[:, :, 1::3]
    cv_ = st6[:, :, 2::3]
    nc.vector.tensor_mul(shc[:, :, 0], cnt[:, :, 0], mu_[:, :, 0])
    nc.vector.tensor_mul(w16[:, 0:8], cnt[:, :, 1], mu_[:, :, 1])
    nc.vector.tensor_add(shc[:, :, 0], shc[:, :, 0], w16[:, 0:8])
    nc.vector.tensor_mul(w16[:, 0:8], mu_[:, :, 0], mu_[:, :, 0])
    nc.vector.tensor_mul(w16[:, 0:8], w16[:, 0:8], cnt[:, :, 0])
    nc.vector.tensor_add(w16[:, 0:8], w16[:, 0:8], cv_[:, :, 0])
    nc.vector.tensor_mul(w16[:, 8:16], mu_[:, :, 1], mu_[:, :, 1])
    nc.vector.tensor_mul(w16[:, 8:16], w16[:, 8:16], cnt[:, :, 1])
    nc.vector.tensor_add(w16[:, 8:16], w16[:, 8:16], cv_[:, :, 1])
    nc.vector.tensor_add(shc[:, :, 1], w16[:, 0:8], w16[:, 8:16])


def _bn_small(nc, S1, S2, n, eps, g, b, outA, outB, w):
    """outA = g*rsqrt(var+eps); outB = b - outA*mu. DVE-only (Newton rsqrt)."""
    I32 = mybir.dt.int32
    k = S1.shape[1]
    nmu, ex2, t0, t2 = (w[:, 0:k], w[:, k:2 * k], w[:, 2 * k:3 * k],
                        w[:, 3 * k:4 * k])
    nc.vector.tensor_scalar_mul(nmu, S1, -1.0 / n)
    nc.vector.tensor_scalar_mul(ex2, S2, 1.0 / n)
    nc.vector.tensor_mul(t0, nmu, nmu)
    nc.vector.tensor_sub(ex2, ex2, t0)
    if isinstance(eps, float):
        nc.vector.tensor_scalar_add(ex2, ex2, eps)
    else:
        nc.vector.tensor_add(ex2, ex2, eps)
    # rsqrt(ex2) via bit-trick + 3 Newton iterations (rel err < 1e-10)
    yi = t0.bitcast(I32)
    nc.vector.tensor_scalar(out=yi, in0=ex2.bitcast(I32), scalar1=1,
                            scalar2=-1, op0=OP.arith_shift_right,
                            op1=OP.bitwise_xor)
    nc.vector.tensor_scalar_add(yi, yi, 0x5f375a87)
    for _ in range(3):
        nc.vector.tensor_mul(t2, t0, t0)
        nc.vector.tensor_mul(t2, t2, ex2)
        nc.vector.tensor_scalar(out=t2, in0=t2, scalar1=-0.5, scalar2=1.5,
                                op0=OP.mult, op1=OP.add)
        nc.vector.tensor_mul(t0, t0, t2)
    nc.vector.tensor_mul(outA, g, t0)
    for j in range(k):
        nc.vector.scalar_tensor_tensor(
            out=outB[:, j:j + 1], in0=outA[:, j:j + 1], scalar=nmu[:, j:j + 1],
            in1=b[:, j:j + 1], op0=OP.mult, op1=OP.add)


# --------------------------------------------------------------------------
# host wrapper
# --------------------------------------------------------------------------

_NC = None


def _get_module():
    global _NC
    if _NC is None:
        import os
        _NC = build_module(int(os.environ.get("KSTAGE", "9")))
    return _NC


def _to_bf16(a):
    return np.asarray(a, dtype=mybir.dt.np(BF16))


def _host_prep(inputs):
    x = np.asarray(inputs["x"], np.float32)
    haar_weight = np.asarray(inputs["haar_weight"], np.float32)
    conv1_w = np.asarray(inputs["conv1_w"], np.float32)
    conv2_w = np.asarray(inputs["conv2_w"], np.float32)

    selc = np.zeros((P, CL), np.float32)
    selc[np.arange(P), np.arange(P) % CL] = 1.0
    selb = np.ascontiguousarray(selc.T)

    def blockdiag16(w_oi):
        m = np.zeros((P, P), np.float32)
        for g in range(8):
            m[g * 16:(g + 1) * 16, g * 16:(g + 1) * 16] = w_oi.T
        return m

    w1blk = _to_bf16(blockdiag16(conv1_w[:, :, 0, 0]))
    w2blk = _to_bf16(np.stack([blockdiag16(conv2_w[:, :, dy, dx])
                               for dy in range(3) for dx in range(3)]))

    thrv = np.zeros((P, 16), np.float32)
    for bi in range(4):
        thrv[:, 4 * bi:4 * bi + 4] = np.float32(256.0) * np.float32(TAUS[bi])

    in_maps = []
    for dd in range(NCORES):
        c0 = CL * dd
        sl = slice(c0, c0 + CL)
        x_core = np.ascontiguousarray(
            x[:, :, sl].transpose(1, 2, 0, 3, 4)).reshape(P, F)
        wkblk = np.zeros((4, P, P), np.float32)
        for k in range(4):
            wk = haar_weight[4 * k + dd // 2]
            for g in range(8):
                wkblk[k, g * 16:(g + 1) * 16, g * 16:(g + 1) * 16] = wk
        wk_host = np.ascontiguousarray(wkblk.transpose(1, 0, 2)).reshape(P, 4 * P)
        w2_host = np.ascontiguousarray(w2blk.transpose(1, 0, 2)).reshape(P, 9 * P)

        bnp = np.zeros((CL, 21), np.float32)
        bnp[:, 0] = inputs["bn_fwd_g"][sl]
        bnp[:, 1] = inputs["bn_fwd_g"][C + c0:C + c0 + CL]
        bnp[:, 2] = inputs["bn_fwd_b"][sl]
        bnp[:, 3] = inputs["bn_fwd_b"][C + c0:C + c0 + CL]
        gm = np.asarray(inputs["bn_mul_g"], np.float32).reshape(4, C)[:, sl]
        bm = np.asarray(inputs["bn_mul_b"], np.float32).reshape(4, C)[:, sl]
        bnp[:, 4:8] = gm.T
        bnp[:, 8:12] = bm.T
        bnp[:, 12] = inputs["bn_inv_g"][sl]
        bnp[:, 13] = inputs["bn_c1_g"][sl]
        bnp[:, 14] = inputs["bn_c2_g"][sl]
        bnp[:, 15] = inputs["bn_inv_b"][sl]
        bnp[:, 16] = inputs["bn_c1_b"][sl]
        bnp[:, 17] = inputs["bn_c2_b"][sl]
        bnp[:, 18] = 4e-5
        bnp[:, 19] = 1e-5
        bnp[:, 20] = 1e-5

        in_maps.append({
            "xin": x_core,
            "w1blk": w1blk,
            "w2blk": w2_host,
            "wkblk": wk_host,
            "selc": selc,
            "selb": selb,
            "bnp": np.ascontiguousarray(bnp),
            "thrv": thrv,
        })
    return in_maps


def _assemble(results):
    out = np.zeros((T, B, C, H, W), np.float32)
    for dd in range(NCORES):
        oc = np.asarray(results[dd]["out"]).reshape(B, CL, T, H, W)
        out[:, :, CL * dd:CL * (dd + 1)] = oc.transpose(2, 0, 1, 3, 4)
    return out


def kernel(**inputs):
    nc = _get_module()
    in_maps = _host_prep(inputs)
    res = run_bass_kernel_spmd(nc, in_maps, list(range(NCORES)))
    return _assemble(res.results)

